# revision 24
# baseline (speedup 1.0000x reference)
"""ALIGN module kernel for 8 TRN2 NeuronCores (vocab-parallel, host-reduced).

Reference computation (B=4, S=576, Dv=1024, Dl=4096, V=32000):
    x  = vision_feats @ W1_w.T + W1_b          # [T=2304, Dl]
    xn = layernorm(x)                          # over Dl, no affine
    P  = softmax(xn @ W2_w.T, axis=-1)         # [T, V]
    F  = P @ llm_token_embed                   # [T, Dl]

Sharding: vocab dim of W2_w / llm_token_embed split across the 8 cores
(4000 rows each, zero-padded to 4096). Every core computes partial
numerators  N_c = exp(xn @ W2_c.T) @ E_c  and partial denominators
s_c = sum_v exp(...)  for ALL tokens; the host sums the 8 partials and
divides. NO ReduceScatter at all -- partials stream to DRAM as mm2
produces them, so the kernel tail is just the last store.

Token superblocks [256, 768, 768, 512]. Superblock 0 is computed
REPLICATED in stage A on every core (vision @ W1 for 256 tokens costs
~27us of PE and removes the first AllGather from the critical path);
superblocks 1-3 are token-parallel (each core computes its contiguous
1/8 chunk of xn) and AllGathered while phase B runs on earlier blocks.

W1/W1_b are COLUMN-CENTERED on the host so the LN mean is exactly 0:
stage A only needs E[x^2] (ones-vector matmul), Rsqrt, a rank-1
broadcast matmul and one elementwise multiply.

DMA queue separation (the previous version funneled 320MB through the
single qSync HWDGE queue -> head-of-line blocking at superblock
boundaries): W2 streams on qSync (nc.sync) as 0.5MB half-tiles
(bufs=6 -> deep prefetch pipeline); emb (2MB half-chunks, bufs=4),
AllGather-result loads, AllGather-payload stores and w1t/vision
startup loads ride qScalar (nc.scalar, which starts ~8us before the
Sync engine finishes its init preamble); partial-numerator stores go
out on qSync during mm2 when it is otherwise idle; the tiny partial-
denominator stores use the gpsimd SWDGE. Every DMA-trigger
instruction costs ~600ns on its issuing engine, so loads are batched
into as few triggers as ring-pipelining allows.
"""

import os
import sys

for _p in ("/opt/trn_rl_repo", "/root/.axon_site/_ro/trn_rl_repo"):
    if os.path.isdir(_p) and _p not in sys.path:
        sys.path.insert(0, _p)

import numpy as np
import ml_dtypes

from concourse import bass, bacc, mybir, tile
from concourse.bass_utils import run_bass_kernel_spmd

BF16NP = ml_dtypes.bfloat16
F32 = mybir.dt.float32
BF16 = mybir.dt.bfloat16

N_CORES = 8
T = 2304          # total tokens (B*S)
DV = 1024
DL = 4096
V_PAD = 4096      # padded vocab rows per core (4000 real + 96 zero pads)
NVT = V_PAD // 128  # 32 vocab tiles per core
NJ = DL // 128      # 32 contraction tiles
NK = DV // 128      # 8 stage-A contraction tiles
EC = 512          # matmul2 embedding-chunk width
N_EC = DL // EC   # 8 e-chunks

# superblocks: (start, size, per-core chunk size). sb0 replicated (tch=full).
SBS = [(0, 256, 256), (256, 768, 96), (1024, 768, 96), (1792, 512, 64)]
# mm1 moving-dim chunking per superblock (multiples of tch for the
# c-block-major xnt layout; sb0/sb3 fit one PSUM bank per chunk)
SB_CHUNKS = [[(0, 256)], [(0, 384), (384, 384)], [(0, 384), (384, 384)],
             [(0, 512)]]
# local vision column offset of each superblock's share
SH_COL = [0, 256, 352, 448]
XNT_ELEMS = 8 * NJ * 96   # flat per-partition extent of the xnt buffer

_NC_CACHE = None


def build():
    nc = bacc.Bacc("TRN2", target_bir_lowering=False, debug=False,
                   num_devices=N_CORES)
    rg = [list(range(N_CORES))]

    visionT = nc.dram_tensor("visionT", [DV, 512], BF16, kind="ExternalInput")
    w1t = nc.dram_tensor("w1t", [DV, DL], BF16, kind="ExternalInput")
    w1b = nc.dram_tensor("w1b", [128, NJ], F32, kind="ExternalInput")
    # [vt][p][j][vi]: per-partition unit-stride 8KB runs
    w2t = nc.dram_tensor("w2t", [NVT, 128, NJ, 128], BF16, kind="ExternalInput")
    # [e][p][vt][n]: per-partition unit-stride 16KB runs
    emb = nc.dram_tensor("emb", [N_EC, 128, NVT, EC], BF16,
                         kind="ExternalInput")
    ones_v = nc.dram_tensor("ones_v", [128, NVT, 1], BF16, kind="ExternalInput")
    # partial numerators / denominators; host sums over cores and divides
    out2 = nc.dram_tensor("out2", [T, DL], BF16, kind="ExternalOutput")
    out3 = nc.dram_tensor("out3", [T, 1], F32, kind="ExternalOutput")

    from contextlib import ExitStack
    with tile.TileContext(nc) as tc, ExitStack() as ctx:
        consts = ctx.enter_context(tc.tile_pool(name="consts", bufs=1))
        dram = ctx.enter_context(tc.tile_pool(name="dram", bufs=1, space="DRAM"))
        # phase-B-critical pools allocated BEFORE stage A so phase B's first
        # loads/matmuls are not gated on stage-A pool release
        w2_p = ctx.enter_context(tc.tile_pool(name="w2_p", bufs=3))
        xnt_p = ctx.enter_context(tc.tile_pool(name="xnt_p", bufs=1))
        l_ps = ctx.enter_context(tc.tile_pool(name="l_ps", bufs=3, space="PSUM"))

        onesv_sb = consts.tile([128, NVT, 1], BF16)
        nc.sync.dma_start(onesv_sb, ones_v[:])

        # ONE AllGather for all three shares (three serialized AGs kept
        # ring traffic competing with the w2 stream for ~200us). Payload
        # per core: three sections [sh1|sh2|sh3], each [p][j][i] with
        # dl = 128*j + p.
        AG_SEC = [None, 0, DL * 96, DL * 192]   # element offsets
        AG_TOT = DL * 256
        ag_in = dram.tile([AG_TOT], BF16, tag="ag_in", name="ag_in")
        ag_out = dram.tile([N_CORES * AG_TOT], BF16, addr_space="Shared",
                           tag="ag_out", name="ag_out")

        # flat xnt buffers (rotating): custom AP views per layout
        xnt_tiles = {}

        def xnt_alloc(si):
            t = xnt_p.tile([128, XNT_ELEMS], BF16, tag="xnt",
                           name=f"xnt_{si}")
            xnt_tiles[si] = t
            return t

        def xnt_rhs(si, j, c0, cw):
            t = xnt_tiles[si]
            tch = SBS[si][2]
            if si == 0:
                return bass.AP(tensor=t.tensor, offset=t.offset + j * 256 + c0,
                               ap=[list(t.ap[0]), [1, cw]])
            nc_blk = cw // tch
            return bass.AP(
                tensor=t.tensor,
                offset=t.offset + (c0 // tch) * (NJ * tch) + j * tch,
                ap=[list(t.ap[0]), [NJ * tch, nc_blk], [1, tch]])

        # preallocate + start the first w2 tiles right away (qSync, after
        # the small const; before the stage-A loads)
        w2_tiles = []

        def w2_alloc(si, vt):
            # full 1MB tiles: one trigger per tile -- with half-tiles the
            # ~600ns trigger overhead makes supply (2.2us/half) slower
            # than sb0's demand (2.1us/half)
            t = w2_p.tile([128, NJ, 128], BF16, tag="w2",
                          name=f"w2_{si}_{vt}")
            nc.sync.dma_start(t, w2t[vt][:])
            return t

        # ---------------- Stage A
        with ExitStack() as actx:
            sa = actx.enter_context(tc.tile_pool(name="stageA", bufs=1))
            sa2 = actx.enter_context(tc.tile_pool(name="stageA2", bufs=2))
            sq_p = actx.enter_context(tc.tile_pool(name="sq_p", bufs=3))
            psa = actx.enter_context(tc.tile_pool(name="psumA", bufs=1,
                                                  space="PSUM"))

            # single-trigger loads: DMA-trigger instructions cost ~600ns
            # each on the issuing engine, so batch big and split across
            # both HWDGE queues (qSync + qScalar). vision + w1t own the
            # HBM first -- everything in stage A waits on them.
            vt_sb = sa.tile([128, NK, 512], BF16)
            nc.scalar.dma_start(
                vt_sb,
                bass.AP(tensor=visionT.ap().tensor, offset=0,
                        ap=[[512, 128], [512 * 128, NK], [1, 512]]))
            w1t_sb = sa.tile([128, NK, DL], BF16)
            for k in range(NK):
                eng = nc.scalar if k < 4 else nc.sync
                eng.dma_start(w1t_sb[:, k, :],
                              w1t[128 * k:128 * (k + 1), :])
            b_cols = sa.tile([128, NJ], F32)
            nc.sync.dma_start(b_cols, w1b[:])
            onescol = sa.tile([128, 1], BF16)
            nc.vector.memset(onescol, 1.0)
            onesrow_f = sa.tile([1, 128], F32)
            nc.vector.memset(onesrow_f, 1.0)
            eps_sc = sa.tile([1, 1], F32)
            nc.vector.memset(eps_sc, 1e-5)

            # prefetch the first three w2 tiles behind the stage-A loads
            for vt in range(3):
                w2_tiles.append(w2_alloc(0, vt))  # each entry: 2 half-tiles

            def bc(t, rep, tch):
                # [128, tch] -> [128, rep, tch] stride-0 broadcast
                return bass.AP(tensor=t.tensor, offset=t.offset,
                               ap=[list(t.ap[0]), [0, rep], [1, tch]])

            # ONE joint m-loop over all 512 local token columns (sb0's 256
            # replicated tokens + the three shares): N=512 hides LDWEIGHTS
            # completely and there is a single act->square latency chain.
            # sb0's x goes straight into the xnt buffer (bf16); the rstd
            # scale is applied in-place at the end.
            xnt0 = xnt_alloc(0)

            def xnt0_row(m):
                return bass.AP(tensor=xnt0.tensor,
                               offset=xnt0.offset + m * 256,
                               ap=[list(xnt0.ap[0]), [1, 256]])

            xtsh = sa2.tile([128, NJ, 256], BF16, tag="xtsh", name="xtsh",
                            bufs=1)
            # (si, local col0, tch); sb0 is cols 0:256
            RANGES = [(0, 0, 256), (1, 256, 96), (2, 352, 96), (3, 448, 64)]
            # one joint E[x^2] chain over all 512 local token columns;
            # ranges slice the result afterward
            s2p = psa.tile([1, 512], F32, tag="s2", name="s2", bufs=1)
            sqs = {}

            def s2_step(m):
                nc.tensor.matmul(s2p, lhsT=onescol, rhs=sqs[m],
                                 start=(m == 0), stop=(m == NJ - 1))
                if m >= 2:
                    sqs.pop(m - 2)

            for m in range(NJ):
                xp = psa.tile([128, 512], F32, tag="xp", name=f"xp_{m}",
                              bufs=2)
                for k in range(NK):
                    nc.tensor.matmul(
                        xp, lhsT=w1t_sb[:, k, 128 * m:128 * (m + 1)],
                        rhs=vt_sb[:, k, :],
                        start=(k == 0), stop=(k == NK - 1))
                nc.scalar.activation(
                    out=xnt0_row(m), in_=xp[:, :256],
                    func=mybir.ActivationFunctionType.Identity,
                    bias=b_cols[:, m:m + 1])
                nc.scalar.activation(
                    out=xtsh[:, m, :], in_=xp[:, 256:],
                    func=mybir.ActivationFunctionType.Identity,
                    bias=b_cols[:, m:m + 1])
                sq = sq_p.tile([128, 512], BF16, tag="sq", name=f"sq_{m}",
                               bufs=4)
                nc.vector.tensor_mul(out=sq[:, :256], in0=xnt0_row(m),
                                     in1=xnt0_row(m))
                nc.vector.tensor_mul(out=sq[:, 256:], in0=xtsh[:, m, :],
                                     in1=xtsh[:, m, :])
                sqs[m] = sq
                if m >= 2:
                    s2_step(m - 2)
            s2_step(NJ - 2)
            s2_step(NJ - 1)

            # rstd chains for all 4 ranges (pipeline on DVE/ACT)
            rstds = []
            for r, (si, c0, tch) in enumerate(RANGES):
                msq_row = sa2.tile([1, 256], F32, tag="msq",
                                   name=f"msq_{r}", bufs=2)
                nc.vector.tensor_scalar(
                    out=msq_row[:, :tch], in0=s2p[:, c0:c0 + tch],
                    scalar1=1.0 / DL, scalar2=None,
                    op0=mybir.AluOpType.mult)
                sd_row = sa2.tile([1, 256], F32, tag="sd",
                                  name=f"sd_{r}", bufs=2)
                nc.scalar.activation(
                    out=sd_row[:, :tch], in_=msq_row[:, :tch],
                    func=mybir.ActivationFunctionType.Sqrt, bias=eps_sc)
                rstd_row = sa2.tile([1, 256], F32, tag="rstd",
                                    name=f"rstd_{r}", bufs=4)
                nc.vector.reciprocal(out=rstd_row[:, :tch],
                                     in_=sd_row[:, :tch])
                rstds.append(rstd_row)

            def sa_finish(r):
                # broadcast rstd, apply; sb0 multiplies xnt0 in place,
                # shares ship to their AllGather
                si, c0, tch = RANGES[r]
                rstd_row = rstds[r]
                # rides the xp ring (the m-loop is done by now)
                rstdb_p = psa.tile([128, 512], F32, tag="xp",
                                   name=f"rstdb_{r}", bufs=2)
                nc.tensor.matmul(rstdb_p[:, :tch], lhsT=onesrow_f,
                                 rhs=rstd_row[:, :tch])
                rstdb = sa2.tile([128, 256], BF16, tag="rstdb_sb",
                                 name=f"rstdb_sb_{r}", bufs=2)
                nc.vector.tensor_copy(out=rstdb[:, :tch],
                                      in_=rstdb_p[:, :tch])
                if si == 0:
                    for q in range(4):
                        dst = bass.AP(
                            tensor=xnt0.tensor,
                            offset=xnt0.offset + q * 8 * 256,
                            ap=[list(xnt0.ap[0]), [256, 8], [1, 256]])
                        nc.vector.tensor_mul(out=dst, in0=dst,
                                             in1=bc(rstdb, 8, 256))
                    return
                xn_ch = sa2.tile([128, NJ, tch], BF16, tag="xn",
                                 name=f"xn_{si}", bufs=1)
                nc.vector.tensor_mul(out=xn_ch,
                                     in0=xtsh[:, :, c0 - 256:c0 - 256 + tch],
                                     in1=bc(rstdb, NJ, tch))
                # qScalar: gated on the xn_ch multiply (stage-A end); on
                # qSync they would head-of-line-block the w2 stream, on the
                # gpsimd SWDGE they complete too slowly and stall phase-B
                # SBUF reuse of this region
                for q in range(4):
                    nc.scalar.dma_start(
                        bass.AP(tensor=ag_in.tensor,
                                offset=ag_in.offset + AG_SEC[si]
                                + q * 8 * tch,
                                ap=[[NJ * tch, 128], [1, 8 * tch]]),
                        xn_ch[:, 8 * q:8 * (q + 1), :])

            # sb0 first: mm1 can start the moment xnt0 is scaled; the
            # share AllGather fires a few us later under mm1's cover
            for r in (0, 1, 2, 3):
                sa_finish(r)
            nc.gpsimd.collective_compute(
                "AllGather", mybir.AluOpType.bypass, replica_groups=rg,
                ins=[ag_in.opt()], outs=[ag_out.opt()])

        # ---------------- Phase B
        pt_p = ctx.enter_context(tc.tile_pool(name="pt_p", bufs=1))
        eb_p = ctx.enter_context(tc.tile_pool(name="eb_p", bufs=4))
        fs_p = ctx.enter_context(tc.tile_pool(name="fs_p", bufs=2))
        ss_p = ctx.enter_context(tc.tile_pool(name="ss_p", bufs=2))
        s_ps = ctx.enter_context(tc.tile_pool(name="s_ps", bufs=2, space="PSUM"))
        f_ps = ctx.enter_context(tc.tile_pool(name="f_ps", bufs=3, space="PSUM"))

        def make_xnt(si):
            # c-block-major loads from the AG output (128 x 6KB runs per
            # block) on the qScalar queue
            t = xnt_alloc(si)
            tch = SBS[si][2]
            for c in range(N_CORES):
                off = ag_out.offset + c * AG_TOT + AG_SEC[si]
                dst = bass.AP(tensor=t.tensor,
                              offset=t.offset + c * NJ * tch,
                              ap=[list(t.ap[0]), [1, NJ * tch]])
                nc.scalar.dma_start(
                    dst, bass.AP(tensor=ag_out.tensor, offset=off,
                                 ap=[[NJ * tch, 128], [1, NJ * tch]]))

        def eb_alloc_h(si, e, h):
            t = eb_p.tile([128, NVT // 2, EC], BF16, tag="eb",
                          name=f"eb_{si}_{e}_{h}")
            nc.scalar.dma_start(
                t, emb[e][:, (NVT // 2) * h:(NVT // 2) * (h + 1), :])
            return t

        def eb_alloc(si, e):
            # two half-tiles per e-chunk: doubles prefetch pipeline depth
            # at the same SBUF footprint
            return [eb_alloc_h(si, e, 0), eb_alloc_h(si, e, 1)]

        for si, (sb0, sbn, tch) in enumerate(SBS):
            n_tt = sbn // 128
            # prefetch this superblock's first two e-chunks (qScalar).
            # During sb0's mm1 the w2 stream needs ~240GB/s of HBM, so
            # there the 8MB eb prefetch is spread over vt 12..24 instead
            # of bursting at the top.
            ebs = {}
            if si > 0:
                ebs = {0: eb_alloc(si, 0), 1: eb_alloc(si, 1)}
            else:
                ebs = {0: [None, None], 1: [None, None]}

            # matmul1: logitsT per v-tile, exp -> pt
            pt = pt_p.tile([128, NVT, 768], BF16, tag="pt", name=f"pt_{si}")
            for vt in range(NVT):
                if si == 0 and vt >= 16 and vt % 4 == 0 and vt <= 28:
                    i = (vt - 16) // 4
                    ebs[i // 2][i % 2] = eb_alloc_h(si, i // 2, i % 2)
                if w2_tiles:
                    w2sb = w2_tiles.pop(0)
                else:
                    w2sb = w2_alloc(si, vt)
                for c0, cw in SB_CHUNKS[si]:
                    lp = l_ps.tile([128, 512], F32, tag="lp",
                                   name=f"lp_{si}_{vt}_{c0}")
                    for j in range(NJ):
                        nc.tensor.matmul(
                            lp[:, :cw], lhsT=w2sb[:, j, :],
                            rhs=xnt_rhs(si, j, c0, cw),
                            start=(j == 0), stop=(j == NJ - 1))
                    nc.scalar.activation(
                        out=pt[:, vt, c0:c0 + cw], in_=lp[:, :cw],
                        func=mybir.ActivationFunctionType.Exp)

            # partial softmax denominators for this superblock -> out3
            for tt in range(n_tt):
                sp = s_ps.tile([128, 1], F32, tag="sp", name=f"sp_{si}_{tt}")
                for vt in range(NVT):
                    nc.tensor.matmul(
                        sp, lhsT=pt[:, vt, 128 * tt:128 * (tt + 1)],
                        rhs=onesv_sb[:, vt, :],
                        start=(vt == 0), stop=(vt == NVT - 1))
                ss = ss_p.tile([128, 1], F32, tag="ss", name=f"ss_{si}_{tt}")
                nc.scalar.activation(
                    out=ss, in_=sp,
                    func=mybir.ActivationFunctionType.Identity)
                nc.gpsimd.dma_start(
                    out3[sb0 + 128 * tt:sb0 + 128 * (tt + 1), :], ss)

            # prefetch the next superblock's first w2 tiles now: qSync is
            # otherwise idle during mm2, and at the boundary the xnt loads
            # occupy qScalar
            if si + 1 < len(SBS):
                for vt in range(3):
                    w2_tiles.append(w2_alloc(si + 1, vt))

            # matmul2: partial F = pt.T @ emb per e-chunk -> out2 (bf16)
            for e in range(N_EC):
                eb = ebs.pop(e)
                for tt in range(n_tt):
                    fp = f_ps.tile([128, EC], F32, tag="fp",
                                   name=f"fp_{si}_{e}_{tt}")
                    for vt in range(NVT):
                        nc.tensor.matmul(
                            fp, lhsT=pt[:, vt, 128 * tt:128 * (tt + 1)],
                            rhs=eb[vt // (NVT // 2)][:, vt % (NVT // 2), :],
                            start=(vt == 0), stop=(vt == NVT - 1))
                    fs = fs_p.tile([128, EC], BF16, tag="fs",
                                   name=f"fs_{si}_{e}_{tt}")
                    nc.scalar.activation(
                        out=fs, in_=fp,
                        func=mybir.ActivationFunctionType.Identity)
                    # qSync is idle during mm2 (w2 streams only during mm1)
                    nc.sync.dma_start(
                        out2[sb0 + 128 * tt:sb0 + 128 * (tt + 1),
                             EC * e:EC * (e + 1)], fs)
                # issue the e+2 prefetch AFTER this iteration's fs stores:
                # its WAR wait (on this iteration's chains) must not
                # head-of-line-block the stores on the qScalar FIFO
                if e + 2 < N_EC:
                    ebs[e + 2] = eb_alloc(si, e + 2)
                if e == N_EC - 1 and si + 1 < len(SBS):
                    # next superblock's xnt loads LAST on qScalar: they wait
                    # on the AllGather, and a late AG (collective-barrier
                    # launch skew can delay AGs by 100s of us) must never
                    # block eb prefetches queued behind them
                    make_xnt(si + 1)

    nc.compile()
    return nc


def _get_nc():
    global _NC_CACHE
    if _NC_CACHE is None:
        _NC_CACHE = build()
    return _NC_CACHE


def _prep_in_maps(vision_feats, W1_w, W1_b, W2_w, llm_token_embed):
    vf = np.ascontiguousarray(np.asarray(vision_feats, np.float32)).reshape(
        T, DV)
    W1 = np.asarray(W1_w, np.float32)
    b1 = np.asarray(W1_b, np.float32).reshape(DL)
    # column-center W1/b over the Dl output dim: makes the LN mean exactly 0
    W1 = W1 - W1.mean(axis=0, keepdims=True)
    b1 = np.ascontiguousarray((b1 - b1.mean()).reshape(NJ, 128).T)
    W2 = np.asarray(W2_w, np.float32)
    E = np.asarray(llm_token_embed, np.float32)

    w1t = np.ascontiguousarray(W1.T).astype(BF16NP)
    v_loc = 32000 // N_CORES
    in_maps = []
    for c in range(N_CORES):
        # vision cols: [sb0 all 256 | own sb1 share | own sb2 | own sb3]
        tok = np.concatenate(
            [np.arange(0, 256)]
            + [np.arange(sb0 + tch * c, sb0 + tch * (c + 1))
               for sb0, _, tch in SBS[1:]])
        vT = np.ascontiguousarray(vf[tok].T).astype(BF16NP)
        w2p = np.zeros((V_PAD, DL), np.float32)
        w2p[:v_loc] = W2[v_loc * c:v_loc * (c + 1)]
        # [vt, p, j, vi] with p = d % 128, j = d // 128, vi = v % 128
        w2tt = w2p.T.reshape(NJ, 128, NVT, 128).transpose(2, 1, 0, 3).astype(
            BF16NP)
        ep = np.zeros((V_PAD, DL), np.float32)
        ep[:v_loc] = E[v_loc * c:v_loc * (c + 1)]
        # [e, p, vt, n] with p = v % 128, vt = v // 128, n = d % EC
        ebt = ep.reshape(NVT, 128, N_EC, EC).transpose(2, 1, 0, 3).astype(
            BF16NP)
        onesv = np.zeros((128, NVT, 1), np.float32)
        for vt in range(NVT):
            for p in range(128):
                if 128 * vt + p < v_loc:
                    onesv[p, vt, 0] = 1.0
        in_maps.append({
            "visionT": vT,
            "w1t": w1t,
            "w1b": b1,
            "w2t": np.ascontiguousarray(w2tt),
            "emb": np.ascontiguousarray(ebt),
            "ones_v": onesv.astype(BF16NP),
        })
    return in_maps


def run_on_cores(in_maps, trace=False, **kwargs):
    nc = _get_nc()
    return run_bass_kernel_spmd(nc, in_maps, core_ids=list(range(N_CORES)),
                                trace=trace, **kwargs)


def assemble(core_results):
    num = np.zeros((T, DL), np.float32)
    den = np.zeros((T, 1), np.float32)
    for c in range(N_CORES):
        num += np.asarray(core_results[c]["out2"]).astype(np.float32)
        den += np.asarray(core_results[c]["out3"])
    return (num / den).reshape(4, 576, DL)


def kernel(**inputs):
    in_maps = _prep_in_maps(**inputs)
    res = run_on_cores(in_maps)
    return assemble(res.results)


# revision 25
# speedup vs baseline: 1.0120x; 1.0120x over previous
"""ALIGN module kernel for 8 TRN2 NeuronCores (vocab-parallel, host-reduced).

Reference computation (B=4, S=576, Dv=1024, Dl=4096, V=32000):
    x  = vision_feats @ W1_w.T + W1_b          # [T=2304, Dl]
    xn = layernorm(x)                          # over Dl, no affine
    P  = softmax(xn @ W2_w.T, axis=-1)         # [T, V]
    F  = P @ llm_token_embed                   # [T, Dl]

Sharding: vocab dim of W2_w / llm_token_embed split across the 8 cores
(4000 rows each, zero-padded to 4096). Every core computes partial
numerators  N_c = exp(xn @ W2_c.T) @ E_c  and partial denominators
s_c = sum_v exp(...)  for ALL tokens; the host sums the 8 partials and
divides. NO ReduceScatter at all -- partials stream to DRAM as mm2
produces them, so the kernel tail is just the last store.

Token superblocks [256, 768, 768, 512]. Superblock 0 is computed
REPLICATED in stage A on every core (vision @ W1 for 256 tokens costs
~27us of PE and removes the first AllGather from the critical path);
superblocks 1-3 are token-parallel (each core computes its contiguous
1/8 chunk of xn) and AllGathered while phase B runs on earlier blocks.

W1/W1_b are COLUMN-CENTERED on the host so the LN mean is exactly 0:
stage A only needs E[x^2] (ones-vector matmul), Rsqrt, a rank-1
broadcast matmul and one elementwise multiply.

DMA queue separation (the previous version funneled 320MB through the
single qSync HWDGE queue -> head-of-line blocking at superblock
boundaries): W2 streams on qSync (nc.sync) as 0.5MB half-tiles
(bufs=6 -> deep prefetch pipeline); emb (2MB half-chunks, bufs=4),
AllGather-result loads, AllGather-payload stores and w1t/vision
startup loads ride qScalar (nc.scalar, which starts ~8us before the
Sync engine finishes its init preamble); partial-numerator stores go
out on qSync during mm2 when it is otherwise idle; the tiny partial-
denominator stores use the gpsimd SWDGE. Every DMA-trigger
instruction costs ~600ns on its issuing engine, so loads are batched
into as few triggers as ring-pipelining allows.
"""

import os
import sys

for _p in ("/opt/trn_rl_repo", "/root/.axon_site/_ro/trn_rl_repo"):
    if os.path.isdir(_p) and _p not in sys.path:
        sys.path.insert(0, _p)

import numpy as np
import ml_dtypes

from concourse import bass, bacc, mybir, tile
from concourse.bass_utils import run_bass_kernel_spmd

BF16NP = ml_dtypes.bfloat16
F32 = mybir.dt.float32
BF16 = mybir.dt.bfloat16

N_CORES = 8
T = 2304          # total tokens (B*S)
DV = 1024
DL = 4096
V_PAD = 4096      # padded vocab rows per core (4000 real + 96 zero pads)
NVT = V_PAD // 128  # 32 vocab tiles per core
NJ = DL // 128      # 32 contraction tiles
NK = DV // 128      # 8 stage-A contraction tiles
EC = 512          # matmul2 embedding-chunk width
N_EC = DL // EC   # 8 e-chunks

# superblocks: (start, size, per-core chunk size). sb0 replicated (tch=full).
SBS = [(0, 256, 256), (256, 768, 96), (1024, 768, 96), (1792, 512, 64)]
# mm1 moving-dim chunking per superblock (multiples of tch for the
# c-block-major xnt layout; sb0/sb3 fit one PSUM bank per chunk)
SB_CHUNKS = [[(0, 256)], [(0, 384), (384, 384)], [(0, 384), (384, 384)],
             [(0, 512)]]
# local vision column offset of each superblock's share
SH_COL = [0, 256, 352, 448]
XNT_ELEMS = 8 * NJ * 96   # flat per-partition extent of the xnt buffer

_NC_CACHE = None


def build():
    nc = bacc.Bacc("TRN2", target_bir_lowering=False, debug=False,
                   num_devices=N_CORES)
    rg = [list(range(N_CORES))]

    visionT = nc.dram_tensor("visionT", [DV, 512], BF16, kind="ExternalInput")
    w1t = nc.dram_tensor("w1t", [DV, DL], BF16, kind="ExternalInput")
    w1b = nc.dram_tensor("w1b", [128, NJ], F32, kind="ExternalInput")
    # [vt][p][j][vi]: per-partition unit-stride 8KB runs
    w2t = nc.dram_tensor("w2t", [NVT, 128, NJ, 128], BF16, kind="ExternalInput")
    # [e][p][vt][n]: per-partition unit-stride 16KB runs
    emb = nc.dram_tensor("emb", [N_EC, 128, NVT, EC], BF16,
                         kind="ExternalInput")
    ones_v = nc.dram_tensor("ones_v", [128, NVT, 1], BF16, kind="ExternalInput")
    # partial numerators / denominators; host sums over cores and divides
    out2 = nc.dram_tensor("out2", [T, DL], BF16, kind="ExternalOutput")
    out3 = nc.dram_tensor("out3", [T, 1], F32, kind="ExternalOutput")

    from contextlib import ExitStack
    with tile.TileContext(nc) as tc, ExitStack() as ctx:
        consts = ctx.enter_context(tc.tile_pool(name="consts", bufs=1))
        dram = ctx.enter_context(tc.tile_pool(name="dram", bufs=1, space="DRAM"))
        # phase-B-critical pools allocated BEFORE stage A so phase B's first
        # loads/matmuls are not gated on stage-A pool release
        w2_p = ctx.enter_context(tc.tile_pool(name="w2_p", bufs=3))
        xnt_p = ctx.enter_context(tc.tile_pool(name="xnt_p", bufs=1))
        l_ps = ctx.enter_context(tc.tile_pool(name="l_ps", bufs=3, space="PSUM"))

        onesv_sb = consts.tile([128, NVT, 1], BF16)
        nc.sync.dma_start(onesv_sb, ones_v[:])

        # ONE AllGather for all three shares (three serialized AGs kept
        # ring traffic competing with the w2 stream for ~200us). Payload
        # per core: three sections [sh1|sh2|sh3], each [p][j][i] with
        # dl = 128*j + p.
        AG_SEC = [None, 0, DL * 96, DL * 192]   # element offsets
        AG_TOT = DL * 256
        ag_in = dram.tile([AG_TOT], BF16, tag="ag_in", name="ag_in")
        ag_out = dram.tile([N_CORES * AG_TOT], BF16, addr_space="Shared",
                           tag="ag_out", name="ag_out")

        # flat xnt buffers (rotating): custom AP views per layout
        xnt_tiles = {}

        def xnt_alloc(si):
            t = xnt_p.tile([128, XNT_ELEMS], BF16, tag="xnt",
                           name=f"xnt_{si}")
            xnt_tiles[si] = t
            return t

        def xnt_rhs(si, j, c0, cw):
            t = xnt_tiles[si]
            tch = SBS[si][2]
            if si == 0:
                return bass.AP(tensor=t.tensor, offset=t.offset + j * 256 + c0,
                               ap=[list(t.ap[0]), [1, cw]])
            nc_blk = cw // tch
            return bass.AP(
                tensor=t.tensor,
                offset=t.offset + (c0 // tch) * (NJ * tch) + j * tch,
                ap=[list(t.ap[0]), [NJ * tch, nc_blk], [1, tch]])

        # preallocate + start the first w2 tiles right away (qSync, after
        # the small const; before the stage-A loads)
        w2_tiles = []

        def w2_alloc(si, vt):
            # full 1MB tiles: one trigger per tile -- with half-tiles the
            # ~600ns trigger overhead makes supply (2.2us/half) slower
            # than sb0's demand (2.1us/half). During sb0's mm1 the stream
            # needs ~240GB/s while the share-AllGather's ring traffic is
            # also in flight, so alternate queues there.
            t = w2_p.tile([128, NJ, 128], BF16, tag="w2",
                          name=f"w2_{si}_{vt}")
            eng = nc.scalar if (si == 0 and vt % 2) else nc.sync
            eng.dma_start(t, w2t[vt][:])
            return t

        # ---------------- Stage A
        with ExitStack() as actx:
            sa = actx.enter_context(tc.tile_pool(name="stageA", bufs=1))
            sa2 = actx.enter_context(tc.tile_pool(name="stageA2", bufs=2))
            sq_p = actx.enter_context(tc.tile_pool(name="sq_p", bufs=3))
            psa = actx.enter_context(tc.tile_pool(name="psumA", bufs=1,
                                                  space="PSUM"))

            # single-trigger loads: DMA-trigger instructions cost ~600ns
            # each on the issuing engine, so batch big and split across
            # both HWDGE queues (qSync + qScalar). vision + w1t own the
            # HBM first -- everything in stage A waits on them.
            vt_sb = sa.tile([128, NK, 512], BF16)
            nc.scalar.dma_start(
                vt_sb,
                bass.AP(tensor=visionT.ap().tensor, offset=0,
                        ap=[[512, 128], [512 * 128, NK], [1, 512]]))
            w1t_sb = sa.tile([128, NK, DL], BF16)
            for k in range(NK):
                eng = nc.scalar if k < 4 else nc.sync
                eng.dma_start(w1t_sb[:, k, :],
                              w1t[128 * k:128 * (k + 1), :])
            b_cols = sa.tile([128, NJ], F32)
            nc.sync.dma_start(b_cols, w1b[:])
            onescol = sa.tile([128, 1], BF16)
            nc.vector.memset(onescol, 1.0)
            onesrow_f = sa.tile([1, 128], F32)
            nc.vector.memset(onesrow_f, 1.0)
            eps_sc = sa.tile([1, 1], F32)
            nc.vector.memset(eps_sc, 1e-5)

            # prefetch the first three w2 tiles behind the stage-A loads
            for vt in range(3):
                w2_tiles.append(w2_alloc(0, vt))  # each entry: 2 half-tiles

            def bc(t, rep, tch):
                # [128, tch] -> [128, rep, tch] stride-0 broadcast
                return bass.AP(tensor=t.tensor, offset=t.offset,
                               ap=[list(t.ap[0]), [0, rep], [1, tch]])

            # ONE joint m-loop over all 512 local token columns (sb0's 256
            # replicated tokens + the three shares): N=512 hides LDWEIGHTS
            # completely and there is a single act->square latency chain.
            # sb0's x goes straight into the xnt buffer (bf16); the rstd
            # scale is applied in-place at the end.
            xnt0 = xnt_alloc(0)

            def xnt0_row(m):
                return bass.AP(tensor=xnt0.tensor,
                               offset=xnt0.offset + m * 256,
                               ap=[list(xnt0.ap[0]), [1, 256]])

            xtsh = sa2.tile([128, NJ, 256], BF16, tag="xtsh", name="xtsh",
                            bufs=1)
            # (si, local col0, tch); sb0 is cols 0:256
            RANGES = [(0, 0, 256), (1, 256, 96), (2, 352, 96), (3, 448, 64)]
            # one joint E[x^2] chain over all 512 local token columns;
            # ranges slice the result afterward
            s2p = psa.tile([1, 512], F32, tag="s2", name="s2", bufs=1)
            sqs = {}

            def s2_step(m):
                nc.tensor.matmul(s2p, lhsT=onescol, rhs=sqs[m],
                                 start=(m == 0), stop=(m == NJ - 1))
                if m >= 2:
                    sqs.pop(m - 2)

            for m in range(NJ):
                xp = psa.tile([128, 512], F32, tag="xp", name=f"xp_{m}",
                              bufs=2)
                for k in range(NK):
                    nc.tensor.matmul(
                        xp, lhsT=w1t_sb[:, k, 128 * m:128 * (m + 1)],
                        rhs=vt_sb[:, k, :],
                        start=(k == 0), stop=(k == NK - 1))
                nc.scalar.activation(
                    out=xnt0_row(m), in_=xp[:, :256],
                    func=mybir.ActivationFunctionType.Identity,
                    bias=b_cols[:, m:m + 1])
                nc.scalar.activation(
                    out=xtsh[:, m, :], in_=xp[:, 256:],
                    func=mybir.ActivationFunctionType.Identity,
                    bias=b_cols[:, m:m + 1])
                sq = sq_p.tile([128, 512], BF16, tag="sq", name=f"sq_{m}",
                               bufs=4)
                nc.vector.tensor_mul(out=sq[:, :256], in0=xnt0_row(m),
                                     in1=xnt0_row(m))
                nc.vector.tensor_mul(out=sq[:, 256:], in0=xtsh[:, m, :],
                                     in1=xtsh[:, m, :])
                sqs[m] = sq
                if m >= 2:
                    s2_step(m - 2)
            s2_step(NJ - 2)
            s2_step(NJ - 1)

            # rstd chains for all 4 ranges (pipeline on DVE/ACT)
            rstds = []
            for r, (si, c0, tch) in enumerate(RANGES):
                msq_row = sa2.tile([1, 256], F32, tag="msq",
                                   name=f"msq_{r}", bufs=2)
                nc.vector.tensor_scalar(
                    out=msq_row[:, :tch], in0=s2p[:, c0:c0 + tch],
                    scalar1=1.0 / DL, scalar2=None,
                    op0=mybir.AluOpType.mult)
                sd_row = sa2.tile([1, 256], F32, tag="sd",
                                  name=f"sd_{r}", bufs=2)
                nc.scalar.activation(
                    out=sd_row[:, :tch], in_=msq_row[:, :tch],
                    func=mybir.ActivationFunctionType.Sqrt, bias=eps_sc)
                rstd_row = sa2.tile([1, 256], F32, tag="rstd",
                                    name=f"rstd_{r}", bufs=4)
                nc.vector.reciprocal(out=rstd_row[:, :tch],
                                     in_=sd_row[:, :tch])
                rstds.append(rstd_row)

            def sa_finish(r):
                # broadcast rstd, apply; sb0 multiplies xnt0 in place,
                # shares ship to their AllGather
                si, c0, tch = RANGES[r]
                rstd_row = rstds[r]
                # rides the xp ring (the m-loop is done by now)
                rstdb_p = psa.tile([128, 512], F32, tag="xp",
                                   name=f"rstdb_{r}", bufs=2)
                nc.tensor.matmul(rstdb_p[:, :tch], lhsT=onesrow_f,
                                 rhs=rstd_row[:, :tch])
                rstdb = sa2.tile([128, 256], BF16, tag="rstdb_sb",
                                 name=f"rstdb_sb_{r}", bufs=2)
                nc.vector.tensor_copy(out=rstdb[:, :tch],
                                      in_=rstdb_p[:, :tch])
                if si == 0:
                    for q in range(4):
                        dst = bass.AP(
                            tensor=xnt0.tensor,
                            offset=xnt0.offset + q * 8 * 256,
                            ap=[list(xnt0.ap[0]), [256, 8], [1, 256]])
                        nc.vector.tensor_mul(out=dst, in0=dst,
                                             in1=bc(rstdb, 8, 256))
                    return
                xn_ch = sa2.tile([128, NJ, tch], BF16, tag="xn",
                                 name=f"xn_{si}", bufs=1)
                nc.vector.tensor_mul(out=xn_ch,
                                     in0=xtsh[:, :, c0 - 256:c0 - 256 + tch],
                                     in1=bc(rstdb, NJ, tch))
                # qScalar: gated on the xn_ch multiply (stage-A end); on
                # qSync they would head-of-line-block the w2 stream, on the
                # gpsimd SWDGE they complete too slowly and stall phase-B
                # SBUF reuse of this region
                for q in range(4):
                    nc.scalar.dma_start(
                        bass.AP(tensor=ag_in.tensor,
                                offset=ag_in.offset + AG_SEC[si]
                                + q * 8 * tch,
                                ap=[[NJ * tch, 128], [1, 8 * tch]]),
                        xn_ch[:, 8 * q:8 * (q + 1), :])

            # sb0 first: mm1 can start the moment xnt0 is scaled; the
            # share AllGather fires a few us later under mm1's cover
            for r in (0, 1, 2, 3):
                sa_finish(r)
            nc.gpsimd.collective_compute(
                "AllGather", mybir.AluOpType.bypass, replica_groups=rg,
                ins=[ag_in.opt()], outs=[ag_out.opt()])

        # ---------------- Phase B
        pt_p = ctx.enter_context(tc.tile_pool(name="pt_p", bufs=1))
        eb_p = ctx.enter_context(tc.tile_pool(name="eb_p", bufs=4))
        fs_p = ctx.enter_context(tc.tile_pool(name="fs_p", bufs=2))
        ss_p = ctx.enter_context(tc.tile_pool(name="ss_p", bufs=2))
        s_ps = ctx.enter_context(tc.tile_pool(name="s_ps", bufs=2, space="PSUM"))
        f_ps = ctx.enter_context(tc.tile_pool(name="f_ps", bufs=3, space="PSUM"))

        def make_xnt(si):
            # c-block-major loads from the AG output (128 x 6KB runs per
            # block) on the qScalar queue
            t = xnt_alloc(si)
            tch = SBS[si][2]
            for c in range(N_CORES):
                off = ag_out.offset + c * AG_TOT + AG_SEC[si]
                dst = bass.AP(tensor=t.tensor,
                              offset=t.offset + c * NJ * tch,
                              ap=[list(t.ap[0]), [1, NJ * tch]])
                nc.scalar.dma_start(
                    dst, bass.AP(tensor=ag_out.tensor, offset=off,
                                 ap=[[NJ * tch, 128], [1, NJ * tch]]))

        def eb_alloc_h(si, e, h):
            t = eb_p.tile([128, NVT // 2, EC], BF16, tag="eb",
                          name=f"eb_{si}_{e}_{h}")
            nc.scalar.dma_start(
                t, emb[e][:, (NVT // 2) * h:(NVT // 2) * (h + 1), :])
            return t

        def eb_alloc(si, e):
            # two half-tiles per e-chunk: doubles prefetch pipeline depth
            # at the same SBUF footprint
            return [eb_alloc_h(si, e, 0), eb_alloc_h(si, e, 1)]

        for si, (sb0, sbn, tch) in enumerate(SBS):
            n_tt = sbn // 128
            # prefetch this superblock's first two e-chunks (qScalar).
            # During sb0's mm1 the w2 stream needs ~240GB/s of HBM, so
            # there the 8MB eb prefetch is spread over vt 12..24 instead
            # of bursting at the top.
            ebs = {}
            if si > 0:
                ebs = {0: eb_alloc(si, 0), 1: eb_alloc(si, 1)}
            else:
                ebs = {0: [None, None], 1: [None, None]}

            # matmul1: logitsT per v-tile, exp -> pt
            pt = pt_p.tile([128, NVT, 768], BF16, tag="pt", name=f"pt_{si}")
            for vt in range(NVT):
                if si == 0 and vt >= 16 and vt % 4 == 0 and vt <= 28:
                    i = (vt - 16) // 4
                    ebs[i // 2][i % 2] = eb_alloc_h(si, i // 2, i % 2)
                if w2_tiles:
                    w2sb = w2_tiles.pop(0)
                else:
                    w2sb = w2_alloc(si, vt)
                for c0, cw in SB_CHUNKS[si]:
                    lp = l_ps.tile([128, 512], F32, tag="lp",
                                   name=f"lp_{si}_{vt}_{c0}")
                    for j in range(NJ):
                        nc.tensor.matmul(
                            lp[:, :cw], lhsT=w2sb[:, j, :],
                            rhs=xnt_rhs(si, j, c0, cw),
                            start=(j == 0), stop=(j == NJ - 1))
                    nc.scalar.activation(
                        out=pt[:, vt, c0:c0 + cw], in_=lp[:, :cw],
                        func=mybir.ActivationFunctionType.Exp)

            # partial softmax denominators for this superblock -> out3
            for tt in range(n_tt):
                sp = s_ps.tile([128, 1], F32, tag="sp", name=f"sp_{si}_{tt}")
                for vt in range(NVT):
                    nc.tensor.matmul(
                        sp, lhsT=pt[:, vt, 128 * tt:128 * (tt + 1)],
                        rhs=onesv_sb[:, vt, :],
                        start=(vt == 0), stop=(vt == NVT - 1))
                ss = ss_p.tile([128, 1], F32, tag="ss", name=f"ss_{si}_{tt}")
                nc.scalar.activation(
                    out=ss, in_=sp,
                    func=mybir.ActivationFunctionType.Identity)
                nc.gpsimd.dma_start(
                    out3[sb0 + 128 * tt:sb0 + 128 * (tt + 1), :], ss)

            # prefetch the next superblock's first w2 tiles now: qSync is
            # otherwise idle during mm2, and at the boundary the xnt loads
            # occupy qScalar
            if si + 1 < len(SBS):
                for vt in range(3):
                    w2_tiles.append(w2_alloc(si + 1, vt))

            # matmul2: partial F = pt.T @ emb per e-chunk -> out2 (bf16)
            for e in range(N_EC):
                eb = ebs.pop(e)
                for tt in range(n_tt):
                    fp = f_ps.tile([128, EC], F32, tag="fp",
                                   name=f"fp_{si}_{e}_{tt}")
                    for vt in range(NVT):
                        nc.tensor.matmul(
                            fp, lhsT=pt[:, vt, 128 * tt:128 * (tt + 1)],
                            rhs=eb[vt // (NVT // 2)][:, vt % (NVT // 2), :],
                            start=(vt == 0), stop=(vt == NVT - 1))
                    fs = fs_p.tile([128, EC], BF16, tag="fs",
                                   name=f"fs_{si}_{e}_{tt}")
                    nc.scalar.activation(
                        out=fs, in_=fp,
                        func=mybir.ActivationFunctionType.Identity)
                    # qSync is idle during mm2 (w2 streams only during mm1)
                    nc.sync.dma_start(
                        out2[sb0 + 128 * tt:sb0 + 128 * (tt + 1),
                             EC * e:EC * (e + 1)], fs)
                # issue the e+2 prefetch AFTER this iteration's fs stores:
                # its WAR wait (on this iteration's chains) must not
                # head-of-line-block the stores on the qScalar FIFO
                if e + 2 < N_EC:
                    ebs[e + 2] = eb_alloc(si, e + 2)
                if e == N_EC - 1 and si + 1 < len(SBS):
                    # next superblock's xnt loads LAST on qScalar: they wait
                    # on the AllGather, and a late AG (collective-barrier
                    # launch skew can delay AGs by 100s of us) must never
                    # block eb prefetches queued behind them
                    make_xnt(si + 1)

    nc.compile()
    return nc


def _get_nc():
    global _NC_CACHE
    if _NC_CACHE is None:
        _NC_CACHE = build()
    return _NC_CACHE


def _prep_in_maps(vision_feats, W1_w, W1_b, W2_w, llm_token_embed):
    vf = np.ascontiguousarray(np.asarray(vision_feats, np.float32)).reshape(
        T, DV)
    W1 = np.asarray(W1_w, np.float32)
    b1 = np.asarray(W1_b, np.float32).reshape(DL)
    # column-center W1/b over the Dl output dim: makes the LN mean exactly 0
    W1 = W1 - W1.mean(axis=0, keepdims=True)
    b1 = np.ascontiguousarray((b1 - b1.mean()).reshape(NJ, 128).T)
    W2 = np.asarray(W2_w, np.float32)
    E = np.asarray(llm_token_embed, np.float32)

    w1t = np.ascontiguousarray(W1.T).astype(BF16NP)
    v_loc = 32000 // N_CORES
    in_maps = []
    for c in range(N_CORES):
        # vision cols: [sb0 all 256 | own sb1 share | own sb2 | own sb3]
        tok = np.concatenate(
            [np.arange(0, 256)]
            + [np.arange(sb0 + tch * c, sb0 + tch * (c + 1))
               for sb0, _, tch in SBS[1:]])
        vT = np.ascontiguousarray(vf[tok].T).astype(BF16NP)
        w2p = np.zeros((V_PAD, DL), np.float32)
        w2p[:v_loc] = W2[v_loc * c:v_loc * (c + 1)]
        # [vt, p, j, vi] with p = d % 128, j = d // 128, vi = v % 128
        w2tt = w2p.T.reshape(NJ, 128, NVT, 128).transpose(2, 1, 0, 3).astype(
            BF16NP)
        ep = np.zeros((V_PAD, DL), np.float32)
        ep[:v_loc] = E[v_loc * c:v_loc * (c + 1)]
        # [e, p, vt, n] with p = v % 128, vt = v // 128, n = d % EC
        ebt = ep.reshape(NVT, 128, N_EC, EC).transpose(2, 1, 0, 3).astype(
            BF16NP)
        onesv = np.zeros((128, NVT, 1), np.float32)
        for vt in range(NVT):
            for p in range(128):
                if 128 * vt + p < v_loc:
                    onesv[p, vt, 0] = 1.0
        in_maps.append({
            "visionT": vT,
            "w1t": w1t,
            "w1b": b1,
            "w2t": np.ascontiguousarray(w2tt),
            "emb": np.ascontiguousarray(ebt),
            "ones_v": onesv.astype(BF16NP),
        })
    return in_maps


def run_on_cores(in_maps, trace=False, **kwargs):
    nc = _get_nc()
    return run_bass_kernel_spmd(nc, in_maps, core_ids=list(range(N_CORES)),
                                trace=trace, **kwargs)


def assemble(core_results):
    num = np.zeros((T, DL), np.float32)
    den = np.zeros((T, 1), np.float32)
    for c in range(N_CORES):
        num += np.asarray(core_results[c]["out2"]).astype(np.float32)
        den += np.asarray(core_results[c]["out3"])
    return (num / den).reshape(4, 576, DL)


def kernel(**inputs):
    in_maps = _prep_in_maps(**inputs)
    res = run_on_cores(in_maps)
    return assemble(res.results)
